# revision 16
# baseline (speedup 1.0000x reference)
"""Trainium2 Bass kernel for nn_GatedLinear (gated LoRA-MoE linear layer).

Math (see reference):
  base_out = x @ base_w.T + base_b
  logits   = x @ router_w.T ; top-2 softmax -> dense per-expert gate
  h        = x @ lora_A.T   ; rank_w = repeat(gate*scalings, 16)
  out      = base_out + (h * rank_w) @ lora_B.T

Sharding: pure data-parallel over batch*seq across 8 cores (1024 tokens
per core); all weights replicated. No collectives.

Device-side strategy (v7):
  * x ships ONLY as an fp16 hi/lo split (xh + rh == x to ~2^-23 rel):
    16MB/core, the minimum for a top-2 selection that matches the fp32
    reference; partition-major layout so a 4-ko block is one 1MB DMA
    with 8KB contiguous runs. The fp8 copy for the base matmul is cast
    on device from xh by the scalar (ACT) engine -- 4MB less DMA in
    the phase-1 critical window than shipping it.
  * Router: the two xh terms (xh@rwh, xh@rwr) merge into ONE matmul
    with a 16-wide stationary [rwh|rwr]; the rh term (rh@rwh)
    accumulates into rows 0:8 of the same [16,GT] PSUM tile; the fold
    happens after the token-major transpose along the free axis (the
    BIR verifier rejects partition-offset PSUM reads).
  * Base matmul: fp8e4m3 DoubleRow (weights host-scaled x64, packed
    [k2,2,f]); measured 216ns per 256x128x512 step = the fp8 roofline
    (HAM-warm 2.4GHz, 1 cycle/moving-token). The x64 scale is folded
    into the e8 gate expansion and removed in the bias epilogue.
  * Four "early" base groups: ot=0 runs during phase-1 streaming
    (interleaved per-ko); ot=1 fills the PE while the DVE runs the
    gating chain (its PSUM banks come from the freed logits tiles).
    All four lora_B closes are emitted after the gating chain.
  * DMA queues: per-queue HBM share is ~1/3 of 358 GB/s when all three
    queues are loaded, so xh/rh blocks ALTERNATE between sync and
    gpsimd (neither stream bound by one queue's share); scalar's queue
    carries only small consts so its engine is free to cast; weight
    stream split scalar/gpsimd behind phase 1; fp16 outputs on sync.
  * Output is fp16 [O, tokens] per core (halves output DMA; ~5e-4 rel
    error) and de-transposed/cast to f32 on the host.

PSUM budget during phase 1 (8 banks): 2 logits + 2 h + 2 transpose
scratch + 2 early base groups; the logits banks recycle into 2 more
base groups mid-gating. Phase 2 uses 6 accumulation groups.
"""

from contextlib import ExitStack

import numpy as np


def _ensure_path():
    try:
        import concourse.bass  # noqa: F401
    except ImportError:
        import sys

        for p in ("/opt/trn_rl_repo", "/root/.axon_site/_ro/trn_rl_repo"):
            if p not in sys.path:
                sys.path.insert(0, p)


N_CORES = 8
B, S, D, O = 4, 2048, 4096, 4096
T = B * S              # 8192 tokens total
T_PC = T // N_CORES    # 1024 tokens per core
E = 8                  # experts
RANK = 16
R = E * RANK           # 128 fused rank dim
P = 128
KO = D // P            # 32 k-subtiles of the contraction dim
KO2 = KO // 2          # paired k-subtiles for DoubleRow (256-deep)
OTILES = O // P        # 32 output-feature tiles
TTILE = 512            # tokens per matmul moving operand
NT = T_PC // TTILE     # 2 token tiles per core
GT = 512               # gating token-tile size
NGT = T_PC // GT       # 2 gating tiles
NGC = GT // P          # 4 128-chunks per gating tile
W8_SCALE = 64.0        # base_w std is 1/64; scale into e4m3's sweet spot
FP8_BASE = True        # kept for test.py's sim threshold selection

_prog_cache = {}


def _build_program():
    """Build the single-core SPMD Bass program (same on all 8 cores)."""
    _ensure_path()
    import concourse.bass as bass
    import concourse.mybir as mybir
    import concourse.tile as tile
    from concourse import bacc

    f32 = mybir.dt.float32
    f16 = mybir.dt.float16
    bf16 = mybir.dt.bfloat16
    f8 = mybir.dt.float8e4
    Alu = mybir.AluOpType
    Act = mybir.ActivationFunctionType
    DR = mybir.MatmulPerfMode.DoubleRow

    nc = bacc.Bacc(
        "TRN2",
        target_bir_lowering=False,
        debug=False,
        num_devices=N_CORES,
    )

    # x hi/lo ship partition-major so a 4-ko block is one 1MB DMA with
    # 8KB contiguous runs per partition (~341 GB/s vs ~100 for 2KB runs)
    xh = nc.dram_tensor("xh", [P, KO * T_PC], f16, kind="ExternalInput").ap()
    rh = nc.dram_tensor("rh", [P, KO * T_PC], f16, kind="ExternalInput").ap()
    wt = nc.dram_tensor(
        "wt", [OTILES * P, KO2 * 2 * P], f8, kind="ExternalInput"
    ).ap()
    lb = nc.dram_tensor("lb", [P, O], bf16, kind="ExternalInput").ap()
    ar = nc.dram_tensor("ar", [P, KO * R], f16, kind="ExternalInput").ap()
    rw2 = nc.dram_tensor("rw2", [P, KO * 2 * E], f16, kind="ExternalInput").ap()
    bb = nc.dram_tensor("bb", [O], f32, kind="ExternalInput").ap()
    e8 = nc.dram_tensor("e8", [E, P], f32, kind="ExternalInput").ap()
    idm = nc.dram_tensor("idm", [P, P], f32, kind="ExternalInput").ap()
    yt = nc.dram_tensor("yt", [O, T_PC], f16, kind="ExternalOutput").ap()

    xh_v = xh.rearrange("p (ko t) -> p ko t", t=T_PC)
    rh_v = rh.rearrange("p (ko t) -> p ko t", t=T_PC)
    wt_v = wt.rearrange("(ot p) (k j f) -> p ot k j f", p=P, j=2, f=P)
    ar_v = ar.rearrange("p (ko r) -> p ko r", r=R)          # [128, 32, 128]
    rw2_v = rw2.rearrange("p (ko c) -> p ko c", c=2 * E)    # [128, 32, 16]
    bb_v = bb.rearrange("(ot p) -> p ot", p=P)              # [128, 32]
    yt_v = yt.rearrange("(ot p) t -> p ot t", p=P)          # [128, 32, 1024]

    # x block schedule: small leading blocks so the PE starts early,
    # 1MB blocks once streaming; xh/rh alternate between the sync and
    # gpsimd queues so neither stream is bound by one queue's HBM share
    XBLOCKS = [(0, 1), (1, 1), (2, 2), (4, 4), (8, 4), (12, 4), (16, 4),
               (20, 4), (24, 4), (28, 4)]
    KOBLK = {}
    for bi, (s, n) in enumerate(XBLOCKS):
        for k in range(s, s + n):
            KOBLK[k] = (bi, k - s)

    with tile.TileContext(nc) as tc:
        with (
            tc.tile_pool(name="perm", bufs=1) as pp,
            tc.tile_pool(name="wstream", bufs=6) as wpool,
            tc.tile_pool(name="rring", bufs=3) as rpool,
            tc.tile_pool(name="obuf", bufs=6) as ob,
        ):
            # ---- consts: rw2 + w0 first on scalar (small, land by
            # ~10us); the scalar engine does half the fp8 casts so its
            # remaining issues are few ----
            rw2sb = pp.tile([P, KO, 2 * E], f16)
            nc.scalar.dma_start(rw2sb[:], rw2_v[:])
            w_sb = [None] * OTILES
            w_sb[0] = wpool.tile([P, KO2, 2, P], f8, tag="w", name="w0")
            nc.scalar.dma_start(w_sb[0][:], wt_v[:, 0, :, :, :])

            # resident tiles
            xhsb = pp.tile([P, KO, T_PC], f16)
            x8sb = pp.tile([P, KO, T_PC], f8)
            rgp = pp.tile([P, T_PC], bf16)   # per-rank gates [r, t]
            hwsb = pp.tile([P, T_PC], bf16)  # gated rank activations [r, t]
            lbsb = pp.tile([P, O], bf16)     # lora_B.T resident
            bbsb = pp.tile([P, OTILES], f32)
            arsb = pp.tile([P, KO, R], f16)
            e8sb = pp.tile([E, P], f32)
            idsb = pp.tile([P, P], f32)

            # lora_A chunk 0 ahead of the x stream on gpsimd (the first
            # h matmul needs it early)
            ARC = 8
            nc.gpsimd.dma_start(arsb[:, 0:ARC, :], ar_v[:, 0:ARC, :])

            # x streams striped over ALL THREE queues (HBM arbitration
            # is per-queue round-robin: 2 active queues only reach ~285
            # GB/s, 3 reach ~330); xh and rh blocks rotate so both
            # streams progress in ko order
            QS = [nc.sync, nc.gpsimd, nc.scalar]
            rh_t = [None] * len(XBLOCKS)
            w_sb[1] = wpool.tile([P, KO2, 2, P], f8, tag="w", name="w1")
            w_sb[2] = wpool.tile([P, KO2, 2, P], f8, tag="w", name="w2")
            for bi, (s, n) in enumerate(XBLOCKS):
                ksl = slice(s, s + n)
                rh_t[bi] = rpool.tile(
                    [P, n, T_PC], f16, tag=f"rh{n}", name=f"rh{bi}"
                )
                QS[bi % 3].dma_start(xhsb[:, ksl, :], xh_v[:, ksl, :])
                QS[(bi + 1) % 3].dma_start(rh_t[bi][:], rh_v[:, ksl, :])
                if bi in (3, 5, 7):
                    c = ARC * (bi - 1) // 2
                    QS[(bi + 2) % 3].dma_start(
                        arsb[:, c : c + ARC, :], ar_v[:, c : c + ARC, :]
                    )
                elif bi == 4:
                    # w1 mid-stream: needed by the gating-window filler
                    nc.sync.dma_start(w_sb[1][:], wt_v[:, 1, :, :, :])
                elif bi == 6:
                    nc.gpsimd.dma_start(w_sb[2][:], wt_v[:, 2, :, :, :])
                elif bi == 8:
                    nc.scalar.dma_start(e8sb[:], e8[:])
                    nc.sync.dma_start(idsb[:], idm[:])
            # tails: lora_B/bias + two more weight tiles
            nc.sync.dma_start(lbsb[:], lb[:])
            nc.gpsimd.dma_start(bbsb[:], bb_v[:])
            for ot in (3, 4):
                w_sb[ot] = wpool.tile([P, KO2, 2, P], f8, tag="w", name=f"w{ot}")
                (nc.scalar if ot == 3 else nc.gpsimd).dma_start(
                    w_sb[ot][:], wt_v[:, ot, :, :, :]
                )

            # ---- phase 1: router + h + early base, paced by x arrival ----
            phase1 = ExitStack()
            stack_l = ExitStack()
            gp = phase1.enter_context(tc.tile_pool(name="gtmp", bufs=2))
            ps_h = phase1.enter_context(
                tc.tile_pool(name="ps_h", bufs=2, space="PSUM")
            )
            ps_t = phase1.enter_context(
                tc.tile_pool(name="ps_t", bufs=2, space="PSUM")
            )
            ps_e = phase1.enter_context(
                tc.tile_pool(name="ps_e", bufs=2, space="PSUM")
            )
            # ps_l created LAST so it can be released first (LIFO),
            # freeing its banks for ps_e2 mid-gating
            ps_l = stack_l.enter_context(
                tc.tile_pool(name="ps_l", bufs=NGT, space="PSUM")
            )

            # logits PSUM [16, GT]: rows 0:8 = xh@rwh (+ rh@rwh), rows
            # 8:16 = xh@rwr; folded after the token-major transpose
            lgs_t = [
                ps_l.tile([2 * E, GT], f32, tag="lg", name=f"lg{g}")
                for g in range(NGT)
            ]
            h_t = [
                ps_h.tile([P, TTILE], f32, tag="h", name=f"h{t}")
                for t in range(NT)
            ]
            # early base groups: (ot=0, tt=0) and (ot=0, tt=1)
            acc_e = [
                ps_e.tile([P, TTILE], f32, tag="acce", name=f"acce{t}")
                for t in range(2)
            ]

            for ko in range(KO):
                bi, off = KOBLK[ko]
                for g in range(NGT):
                    gs = slice(g * GT, (g + 1) * GT)

                    def t13(start, stop):
                        nc.tensor.matmul(
                            lgs_t[g][:],
                            lhsT=rw2sb[:, ko, :],
                            rhs=xhsb[:, ko, gs],
                            start=start,
                            stop=stop,
                        )

                    def t2():
                        nc.tensor.matmul(
                            lgs_t[g][:E, :],
                            lhsT=rw2sb[:, ko, :E],
                            rhs=rh_t[bi][:, off, gs],
                            start=False,
                            stop=False,
                        )

                    if ko == 0:
                        t13(True, False)
                        t2()
                    elif ko == KO - 1:
                        t2()
                        t13(False, True)
                    else:
                        t2()
                        t13(False, False)
                for tt in range(NT):
                    ts = slice(tt * TTILE, (tt + 1) * TTILE)
                    nc.tensor.matmul(
                        h_t[tt][:],
                        lhsT=arsb[:, ko, :],
                        rhs=xhsb[:, ko, ts],
                        start=(ko == 0),
                        stop=(ko == KO - 1),
                    )
                # fp8 cast of this chunk, split DVE/ACT (both engines
                # idle in phase 1; all their DMA issues precede the
                # casts in their instruction streams)
                if ko % 2 == 0:
                    nc.vector.tensor_copy(x8sb[:, ko, :], xhsb[:, ko, :])
                else:
                    nc.scalar.activation(
                        x8sb[:, ko, :], xhsb[:, ko, :], Act.Copy
                    )
                # early base DR step after each odd chunk's cast
                if ko % 2 == 1:
                    k2 = ko // 2
                    for tt in range(2):
                        ts = slice(tt * TTILE, (tt + 1) * TTILE)
                        nc.tensor.matmul(
                            acc_e[tt][:],
                            lhsT=w_sb[0][:, k2, :, :],
                            rhs=x8sb[:, 2 * k2 : 2 * k2 + 2, ts],
                            start=(k2 == 0),
                            stop=False,
                            perf_mode=DR,
                        )

            # ---- gating: top-2 softmax -> per-rank gates ----
            # copy both logits tiles out of PSUM first, then free their
            # banks for two more base groups (ot=1) that fill the PE
            # while the DVE runs the gating chain
            lgs16 = []
            for g in range(NGT):
                t_ = gp.tile([2 * E, GT], f32, tag="lgs", name=f"lgs{g}")
                nc.vector.tensor_copy(t_[:], lgs_t[g][:])
                lgs16.append(t_)
            stack_l.close()
            ps_e2 = phase1.enter_context(
                tc.tile_pool(name="ps_e2", bufs=2, space="PSUM")
            )
            acc_e2 = [
                ps_e2.tile([P, TTILE], f32, tag="acce2", name=f"acce2{t}")
                for t in range(2)
            ]

            def dr_fill(acc, ot, tt, k2s):
                ts = slice(tt * TTILE, (tt + 1) * TTILE)
                for k2 in k2s:
                    nc.tensor.matmul(
                        acc[:],
                        lhsT=w_sb[ot][:, k2, :, :],
                        rhs=x8sb[:, 2 * k2 : 2 * k2 + 2, ts],
                        start=(k2 == 0),
                        stop=False,
                        perf_mode=DR,
                    )

            # token-major transpose of the [16, GT] logits (both tiles)
            ltk16s = []
            for g in range(NGT):
                ltk16 = gp.tile([P, NGC, 2 * E], f32, tag="ltk16", name=f"lt16{g}")
                for c in range(NGC):
                    tp = ps_t.tile([P, GT], f32, tag="pt", name="tp")[:, : 2 * E]
                    nc.tensor.transpose(
                        tp[:], lgs16[g][:, c * P : (c + 1) * P],
                        idsb[: 2 * E, : 2 * E],
                    )
                    nc.vector.tensor_copy(ltk16[:, c, :], tp[:])
                ltk16s.append(ltk16)

            # PE filler while the DVE top-2 chain runs
            dr_fill(acc_e2[0], 1, 0, range(KO2))

            gates = []
            for g in range(NGT):
                ltk16 = ltk16s[g]
                ltk = gp.tile([P, NGC, E], f32, tag="ltk", name=f"ltk{g}")
                nc.vector.tensor_tensor(
                    ltk[:], ltk16[:, :, :E], ltk16[:, :, E:], Alu.add
                )
                m1 = gp.tile([P, NGC, 1], f32, tag="m1")
                nc.vector.tensor_reduce(m1[:], ltk[:], mybir.AxisListType.X, Alu.max)
                mask1 = gp.tile([P, NGC, E], f32, tag="mask1")
                nc.vector.tensor_tensor(
                    mask1[:], ltk[:], m1.to_broadcast((P, NGC, E)), Alu.is_equal
                )
                l2 = gp.tile([P, NGC, E], f32, tag="l2")
                nc.vector.scalar_tensor_tensor(
                    l2[:], mask1[:], -1e30, ltk[:], Alu.mult, Alu.add
                )
                m2 = gp.tile([P, NGC, 1], f32, tag="m2")
                nc.vector.tensor_reduce(m2[:], l2[:], mybir.AxisListType.X, Alu.max)
                mask2 = gp.tile([P, NGC, E], f32, tag="mask2")
                nc.vector.tensor_tensor(
                    mask2[:], l2[:], m2.to_broadcast((P, NGC, E)), Alu.is_equal
                )
                dlt = gp.tile([P, NGC, 1], f32, tag="dlt")
                nc.vector.tensor_tensor(dlt[:], m2[:], m1[:], Alu.subtract)
                dlts = gp.tile([P, NGC, 1], f32, tag="dlts")
                nc.vector.tensor_scalar(
                    dlts[:], dlt[:], 1.0 / 64.0, 0.0, Alu.mult, Alu.add
                )
                g2 = gp.tile([P, NGC, 1], f32, tag="g2")
                nc.scalar.activation(g2[:], dlts[:], Act.Sigmoid)
                g1 = gp.tile([P, NGC, 1], f32, tag="g1")
                nc.vector.tensor_scalar(g1[:], g2[:], -1.0, 1.0, Alu.mult, Alu.add)

                gate = gp.tile([P, NGC, E], f32, tag="gate", name=f"gate{g}")
                nc.vector.tensor_tensor(
                    gate[:], mask1[:], g1.to_broadcast((P, NGC, E)), Alu.mult
                )
                gm2 = gp.tile([P, NGC, E], f32, tag="gm2")
                nc.vector.tensor_tensor(
                    gm2[:], mask2[:], g2.to_broadcast((P, NGC, E)), Alu.mult
                )
                nc.vector.tensor_tensor(gate[:], gate[:], gm2[:], Alu.add)
                gates.append(gate)

            # second PE filler group
            dr_fill(acc_e2[1], 1, 1, range(KO2))

            for g in range(NGT):
                gs = slice(g * GT, (g + 1) * GT)
                # transpose gates back to expert-major [8, 512]
                gts = gp.tile([E, GT], f32, tag="gts", name=f"gts{g}")
                for c in range(NGC):
                    tp2 = ps_t.tile([P, GT], f32, tag="pt", name="tp2")[:E, :P]
                    nc.tensor.transpose(tp2[:], gates[g][:, c, :], idsb[:])
                    nc.vector.tensor_copy(gts[:, c * P : (c + 1) * P], tp2[:])

                # expand expert gates (x scaling*64, folded into e8) to
                # the 128 rank slots: RG = e8.T @ gts
                RG = ps_t.tile([P, GT], f32, tag="pt", name="RG")
                nc.tensor.matmul(
                    RG[:], lhsT=e8sb[:], rhs=gts[:], start=True, stop=True
                )
                nc.vector.tensor_copy(rgp[:, gs], RG[:])
                # gated rank activations for this token tile (g == tt)
                nc.vector.tensor_tensor(
                    hwsb[:, gs], h_t[g][:], rgp[:, gs], Alu.mult
                )

            # ---- close the four early groups: lora term + epilogue ----
            for ot, accs in ((0, acc_e), (1, acc_e2)):
                os_ = slice(ot * P, (ot + 1) * P)
                for tt in range(2):
                    ts = slice(tt * TTILE, (tt + 1) * TTILE)
                    nc.tensor.matmul(
                        accs[tt][:],
                        lhsT=lbsb[:, os_],
                        rhs=hwsb[:, ts],
                        start=False,
                        stop=True,
                    )
                    osb = ob.tile([P, TTILE], f16, tag="osb", name="osbe")
                    nc.vector.scalar_tensor_tensor(
                        osb[:],
                        accs[tt][:],
                        1.0 / W8_SCALE,
                        bbsb[:, ot, None].to_broadcast((P, TTILE)),
                        Alu.mult,
                        Alu.add,
                    )
                    (nc.sync if tt == 0 else nc.scalar).dma_start(
                        yt_v[:, ot, ts], osb[:]
                    )

            phase1.close()

            # ---- phase 2: remaining base matmul + fused lora_B ----
            phase2 = ExitStack()
            ps_o = phase2.enter_context(
                tc.tile_pool(name="ps_o", bufs=6, space="PSUM")
            )

            for ot in range(2, OTILES):
                # keep the weight stream ~3 tiles ahead, alternating queues
                pre = ot + 3
                if pre < OTILES and w_sb[pre] is None:
                    w_sb[pre] = wpool.tile(
                        [P, KO2, 2, P], f8, tag="w", name=f"w{pre}"
                    )
                    eng = nc.scalar if pre % 2 == 0 else nc.gpsimd
                    eng.dma_start(w_sb[pre][:], wt_v[:, pre, :, :, :])
                os_ = slice(ot * P, (ot + 1) * P)
                for tt in range(NT):
                    ts = slice(tt * TTILE, (tt + 1) * TTILE)
                    acc = ps_o.tile([P, TTILE], f32, tag="acc")
                    for k2 in range(KO2):
                        nc.tensor.matmul(
                            acc[:],
                            lhsT=w_sb[ot][:, k2, :, :],
                            rhs=x8sb[:, 2 * k2 : 2 * k2 + 2, ts],
                            start=(k2 == 0),
                            stop=False,
                            perf_mode=DR,
                        )
                    nc.tensor.matmul(
                        acc[:],
                        lhsT=lbsb[:, os_],
                        rhs=hwsb[:, ts],
                        start=False,
                        stop=True,
                    )
                    osb = ob.tile([P, TTILE], f16, tag="osb")
                    # acc holds 64x(base+lora); rescale + bias in one op
                    nc.vector.scalar_tensor_tensor(
                        osb[:],
                        acc[:],
                        1.0 / W8_SCALE,
                        bbsb[:, ot, None].to_broadcast((P, TTILE)),
                        Alu.mult,
                        Alu.add,
                    )
                    (nc.sync if tt == 0 else nc.scalar).dma_start(
                        yt_v[:, ot, ts], osb[:]
                    )
            phase2.close()

    nc.compile()
    return nc


def get_program():
    if "nc" not in _prog_cache:
        _prog_cache["nc"] = _build_program()
    return _prog_cache["nc"]


def make_in_maps(x, base_w, base_b, lora_A, lora_B, router_w, scalings):
    """Host-side sharding/layout prep -> per-core input dicts."""
    import ml_dtypes

    x = np.ascontiguousarray(x, dtype=np.float32)
    # partition-major layout [P, KO, T]: per-core 4-ko DMA blocks are
    # 1MB with 8KB contiguous runs per partition
    xt_full = np.ascontiguousarray(
        x.reshape(T, KO, P).transpose(2, 1, 0)
    )  # [P, KO, T]

    # base weights x64 -> e4m3, DoubleRow pair layout [ot,p,k2,j,f]
    wt_host = np.ascontiguousarray(
        (base_w.T.astype(np.float32) * W8_SCALE)
        .reshape(KO2, 2, P, OTILES, P)
        .transpose(3, 2, 0, 1, 4)
        .reshape(OTILES * P, KO2 * 2 * P)
        .astype(ml_dtypes.float8_e4m3)
    )
    lb_host = np.ascontiguousarray(
        lora_B.T.astype(np.float32).astype(ml_dtypes.bfloat16)
    )

    # lora_A.T (unscaled; scaling folded into e8) -> [p, ko*128+r]
    ar_host = np.ascontiguousarray(
        lora_A.T.astype(np.float32)
        .reshape(KO, P, R)
        .transpose(1, 0, 2)
        .reshape(P, KO * R)
        .astype(np.float16)
    )

    # router_w.T x64 -> [p, ko, 16]: cols 0:8 = fp16 hi, 8:16 = fp16 lo
    # (hi + lo == 64*rw to ~2^-24 relative)
    rw64 = np.ascontiguousarray(
        router_w.T.astype(np.float32)
        .reshape(KO, P, E)
        .transpose(1, 0, 2)
    ) * np.float32(64.0)                                  # [P, KO, E]
    rwh_host = rw64.astype(np.float16)
    rwr_host = (rw64 - rwh_host.astype(np.float32)).astype(np.float16)
    rw2_host = np.ascontiguousarray(
        np.concatenate([rwh_host, rwr_host], axis=-1).reshape(P, KO * 2 * E)
    )

    # expert -> rank-slot expansion with per-expert scaling and the x64
    # fp8 weight scale folded in (so the lora matmul accumulates at the
    # same scale as the fp8 base steps)
    e8 = np.zeros((E, P), dtype=np.float32)
    s = np.asarray(scalings, dtype=np.float32) * W8_SCALE
    for e in range(E):
        e8[e, e * RANK : (e + 1) * RANK] = s[e]
    idm = np.eye(P, dtype=np.float32)
    bbf = base_b.astype(np.float32)

    xh_full = xt_full.astype(np.float16)
    rh_full = (xt_full - xh_full.astype(np.float32)).astype(np.float16)

    in_maps = []
    for c in range(N_CORES):
        cs = slice(c * T_PC, (c + 1) * T_PC)
        m = {
            "xh": np.ascontiguousarray(xh_full[:, :, cs]).reshape(P, KO * T_PC),
            "rh": np.ascontiguousarray(rh_full[:, :, cs]).reshape(P, KO * T_PC),
            "wt": wt_host,
            "lb": lb_host,
            "ar": ar_host,
            "rw2": rw2_host,
            "bb": bbf,
            "e8": e8,
            "idm": idm,
        }
        in_maps.append(m)
    return in_maps


def assemble_output(results):
    """Per-core yt [O, T_PC] fp16 -> full [B, S, O] f32."""
    yt_full = np.concatenate(
        [np.asarray(r["yt"]) for r in results], axis=1
    )  # [O, T] fp16
    return np.ascontiguousarray(yt_full.T.astype(np.float32)).reshape(B, S, O)


def kernel(**inputs):
    _ensure_path()
    from concourse.bass_utils import run_bass_kernel_spmd

    assert int(inputs["top_k"]) == 2
    nc = get_program()
    in_maps = make_in_maps(
        inputs["x"],
        inputs["base_w"],
        inputs["base_b"],
        inputs["lora_A"],
        inputs["lora_B"],
        inputs["router_w"],
        inputs["scalings"],
    )
    res = run_bass_kernel_spmd(nc, in_maps, list(range(N_CORES)))
    return assemble_output(res.results)


if __name__ == "__main__":
    # quick smoke: build the program only
    get_program()
    print("program built OK")


# revision 17
# speedup vs baseline: 1.0099x; 1.0099x over previous
"""Trainium2 Bass kernel for nn_GatedLinear (gated LoRA-MoE linear layer).

Math (see reference):
  base_out = x @ base_w.T + base_b
  logits   = x @ router_w.T ; top-2 softmax -> dense per-expert gate
  h        = x @ lora_A.T   ; rank_w = repeat(gate*scalings, 16)
  out      = base_out + (h * rank_w) @ lora_B.T

Sharding: pure data-parallel over batch*seq across 8 cores (1024 tokens
per core); all weights replicated. No collectives.

Device-side strategy (v7):
  * x ships ONLY as an fp16 hi/lo split (xh + rh == x to ~2^-23 rel):
    16MB/core, the minimum for a top-2 selection that matches the fp32
    reference; partition-major layout so a 4-ko block is one 1MB DMA
    with 8KB contiguous runs. The fp8 copy for the base matmul is cast
    on device from xh by the scalar (ACT) engine -- 4MB less DMA in
    the phase-1 critical window than shipping it.
  * Router: the two xh terms (xh@rwh, xh@rwr) merge into ONE matmul
    with a 16-wide stationary [rwh|rwr]; the rh term (rh@rwh)
    accumulates into rows 0:8 of the same [16,GT] PSUM tile; the fold
    happens after the token-major transpose along the free axis (the
    BIR verifier rejects partition-offset PSUM reads).
  * Base matmul: fp8e4m3 DoubleRow (weights host-scaled x64, packed
    [k2,2,f]); measured 216ns per 256x128x512 step = the fp8 roofline
    (HAM-warm 2.4GHz, 1 cycle/moving-token). The x64 scale is folded
    into the e8 gate expansion and removed in the bias epilogue.
  * Four "early" base groups: ot=0 runs during phase-1 streaming
    (interleaved per-ko); ot=1 fills the PE while the DVE runs the
    gating chain (its PSUM banks come from the freed logits tiles).
    All four lora_B closes are emitted after the gating chain.
  * DMA queues: per-queue HBM share is ~1/3 of 358 GB/s when all three
    queues are loaded, so xh/rh blocks ALTERNATE between sync and
    gpsimd (neither stream bound by one queue's share); scalar's queue
    carries only small consts so its engine is free to cast; weight
    stream split scalar/gpsimd behind phase 1; fp16 outputs on sync.
  * Output is fp16 [O, tokens] per core (halves output DMA; ~5e-4 rel
    error) and de-transposed/cast to f32 on the host.

PSUM budget during phase 1 (8 banks): 2 logits + 2 h + 2 transpose
scratch + 2 early base groups; the logits banks recycle into 2 more
base groups mid-gating. Phase 2 uses 6 accumulation groups.
"""

from contextlib import ExitStack

import numpy as np


def _ensure_path():
    try:
        import concourse.bass  # noqa: F401
    except ImportError:
        import sys

        for p in ("/opt/trn_rl_repo", "/root/.axon_site/_ro/trn_rl_repo"):
            if p not in sys.path:
                sys.path.insert(0, p)


N_CORES = 8
B, S, D, O = 4, 2048, 4096, 4096
T = B * S              # 8192 tokens total
T_PC = T // N_CORES    # 1024 tokens per core
E = 8                  # experts
RANK = 16
R = E * RANK           # 128 fused rank dim
P = 128
KO = D // P            # 32 k-subtiles of the contraction dim
KO2 = KO // 2          # paired k-subtiles for DoubleRow (256-deep)
OTILES = O // P        # 32 output-feature tiles
TTILE = 512            # tokens per matmul moving operand
NT = T_PC // TTILE     # 2 token tiles per core
GT = 512               # gating token-tile size
NGT = T_PC // GT       # 2 gating tiles
NGC = GT // P          # 4 128-chunks per gating tile
W8_SCALE = 64.0        # base_w std is 1/64; scale into e4m3's sweet spot
FP8_BASE = True        # kept for test.py's sim threshold selection

_prog_cache = {}


def _build_program():
    """Build the single-core SPMD Bass program (same on all 8 cores)."""
    _ensure_path()
    import concourse.bass as bass
    import concourse.mybir as mybir
    import concourse.tile as tile
    from concourse import bacc

    f32 = mybir.dt.float32
    f16 = mybir.dt.float16
    bf16 = mybir.dt.bfloat16
    f8 = mybir.dt.float8e4
    Alu = mybir.AluOpType
    Act = mybir.ActivationFunctionType
    DR = mybir.MatmulPerfMode.DoubleRow

    nc = bacc.Bacc(
        "TRN2",
        target_bir_lowering=False,
        debug=False,
        num_devices=N_CORES,
    )

    # x hi/lo ship PACKED per-ko ([P, ko, {hi,lo}, t], partition-major):
    # one DMA delivers both halves of a ko-range together (no cross-
    # stream arrival jitter) as a >=1MB transfer with long runs
    xr = nc.dram_tensor(
        "xr", [P, KO * 2 * T_PC], f16, kind="ExternalInput"
    ).ap()
    wt = nc.dram_tensor(
        "wt", [OTILES * P, KO2 * 2 * P], f8, kind="ExternalInput"
    ).ap()
    lb = nc.dram_tensor("lb", [P, O], bf16, kind="ExternalInput").ap()
    ar = nc.dram_tensor("ar", [P, KO * R], f16, kind="ExternalInput").ap()
    rw2 = nc.dram_tensor("rw2", [P, KO * 2 * E], f16, kind="ExternalInput").ap()
    bb = nc.dram_tensor("bb", [O], f32, kind="ExternalInput").ap()
    e8 = nc.dram_tensor("e8", [E, P], f32, kind="ExternalInput").ap()
    idm = nc.dram_tensor("idm", [P, P], f32, kind="ExternalInput").ap()
    yt = nc.dram_tensor("yt", [O, T_PC], f16, kind="ExternalOutput").ap()

    xr_v = xr.rearrange("p (ko j t) -> p ko j t", j=2, t=T_PC)
    wt_v = wt.rearrange("(ot p) (k j f) -> p ot k j f", p=P, j=2, f=P)
    ar_v = ar.rearrange("p (ko r) -> p ko r", r=R)          # [128, 32, 128]
    rw2_v = rw2.rearrange("p (ko c) -> p ko c", c=2 * E)    # [128, 32, 16]
    bb_v = bb.rearrange("(ot p) -> p ot", p=P)              # [128, 32]
    yt_v = yt.rearrange("(ot p) t -> p ot t", p=P)          # [128, 32, 1024]

    # x block schedule: small leading blocks so the PE starts early,
    # then uniform 2-ko (1MB) blocks rotating over the three queues
    XBLOCKS = [(0, 1), (1, 1), (2, 2)] + [(4 + 2 * i, 2) for i in range(14)]
    KOBLK = {}
    for bi, (s, n) in enumerate(XBLOCKS):
        for k in range(s, s + n):
            KOBLK[k] = (bi, k - s)

    with tile.TileContext(nc) as tc:
        with (
            tc.tile_pool(name="perm", bufs=1) as pp,
            tc.tile_pool(name="wstream", bufs=6) as wpool,
            tc.tile_pool(name="rring", bufs=3) as rpool,
            tc.tile_pool(name="obuf", bufs=6) as ob,
        ):
            # ---- consts: rw2 + w0 first on scalar (small, land by
            # ~10us); the scalar engine does half the fp8 casts so its
            # remaining issues are few ----
            rw2sb = pp.tile([P, KO, 2 * E], f16)
            nc.scalar.dma_start(rw2sb[:], rw2_v[:])
            w_sb = [None] * OTILES
            w_sb[0] = wpool.tile([P, KO2, 2, P], f8, tag="w", name="w0")
            nc.scalar.dma_start(w_sb[0][:], wt_v[:, 0, :, :, :])

            # resident tiles (x8 is the only resident copy of x; the
            # fp16 hi/lo ring tiles die as soon as their ko is consumed)
            x8sb = pp.tile([P, KO, T_PC], f8)
            rgp = pp.tile([P, T_PC], bf16)   # per-rank gates [r, t]
            hwsb = pp.tile([P, T_PC], bf16)  # gated rank activations [r, t]
            lbsb = pp.tile([P, O], bf16)     # lora_B.T resident
            bbsb = pp.tile([P, OTILES], f32)
            arsb = pp.tile([P, KO, R], f16)
            e8sb = pp.tile([E, P], f32)
            idsb = pp.tile([P, P], f32)

            # lora_A chunk 0 ahead of the x stream on gpsimd (the first
            # h matmul needs it early)
            ARC = 8
            nc.gpsimd.dma_start(arsb[:, 0:ARC, :], ar_v[:, 0:ARC, :])

            # packed x stream rotating over ALL THREE queues (HBM
            # arbitration is per-queue round-robin: 2 active queues only
            # reach ~285 GB/s, 3 reach ~330); consts slotted on the
            # off-rotation queues
            QS = [nc.sync, nc.gpsimd, nc.scalar]
            xr_t = [None] * len(XBLOCKS)
            w_sb[1] = wpool.tile([P, KO2, 2, P], f8, tag="w", name="w1")
            w_sb[2] = wpool.tile([P, KO2, 2, P], f8, tag="w", name="w2")
            for bi, (s, n) in enumerate(XBLOCKS):
                xr_t[bi] = rpool.tile(
                    [P, n, 2, T_PC], f16, tag=f"xr{n}", name=f"xr{bi}"
                )
                QS[bi % 3].dma_start(xr_t[bi][:], xr_v[:, s : s + n, :, :])
                if bi in (4, 7, 10):
                    c = ARC * ((bi - 1) // 3)
                    QS[(bi + 1) % 3].dma_start(
                        arsb[:, c : c + ARC, :], ar_v[:, c : c + ARC, :]
                    )
                elif bi == 8:
                    # w1 mid-stream: needed by the gating-window filler
                    QS[(bi + 1) % 3].dma_start(w_sb[1][:], wt_v[:, 1, :, :, :])
                elif bi == 12:
                    QS[(bi + 1) % 3].dma_start(w_sb[2][:], wt_v[:, 2, :, :, :])
                elif bi == 14:
                    QS[(bi + 1) % 3].dma_start(e8sb[:], e8[:])
                    QS[(bi + 2) % 3].dma_start(idsb[:], idm[:])
            # tails: lora_B/bias + two more weight tiles
            nc.sync.dma_start(lbsb[:], lb[:])
            nc.gpsimd.dma_start(bbsb[:], bb_v[:])
            for ot in (3, 4):
                w_sb[ot] = wpool.tile([P, KO2, 2, P], f8, tag="w", name=f"w{ot}")
                (nc.scalar if ot == 3 else nc.gpsimd).dma_start(
                    w_sb[ot][:], wt_v[:, ot, :, :, :]
                )

            # ---- phase 1: router + h + early base, paced by x arrival ----
            phase1 = ExitStack()
            stack_l = ExitStack()
            gp = phase1.enter_context(tc.tile_pool(name="gtmp", bufs=2))
            ps_h = phase1.enter_context(
                tc.tile_pool(name="ps_h", bufs=2, space="PSUM")
            )
            ps_t = phase1.enter_context(
                tc.tile_pool(name="ps_t", bufs=2, space="PSUM")
            )
            ps_e = phase1.enter_context(
                tc.tile_pool(name="ps_e", bufs=2, space="PSUM")
            )
            # ps_l created LAST so it can be released first (LIFO),
            # freeing its banks for ps_e2 mid-gating
            ps_l = stack_l.enter_context(
                tc.tile_pool(name="ps_l", bufs=NGT, space="PSUM")
            )

            # logits PSUM [16, GT]: rows 0:8 = xh@rwh (+ rh@rwh), rows
            # 8:16 = xh@rwr; folded after the token-major transpose
            lgs_t = [
                ps_l.tile([2 * E, GT], f32, tag="lg", name=f"lg{g}")
                for g in range(NGT)
            ]
            h_t = [
                ps_h.tile([P, TTILE], f32, tag="h", name=f"h{t}")
                for t in range(NT)
            ]
            # early base groups: (ot=0, tt=0) and (ot=0, tt=1)
            acc_e = [
                ps_e.tile([P, TTILE], f32, tag="acce", name=f"acce{t}")
                for t in range(2)
            ]

            for ko in range(KO):
                bi, off = KOBLK[ko]
                xh_ko = xr_t[bi][:, off, 0, :]
                rh_ko = xr_t[bi][:, off, 1, :]
                for g in range(NGT):
                    gs = slice(g * GT, (g + 1) * GT)

                    def t13(start, stop):
                        nc.tensor.matmul(
                            lgs_t[g][:],
                            lhsT=rw2sb[:, ko, :],
                            rhs=xh_ko[:, gs],
                            start=start,
                            stop=stop,
                        )

                    def t2():
                        nc.tensor.matmul(
                            lgs_t[g][:E, :],
                            lhsT=rw2sb[:, ko, :E],
                            rhs=rh_ko[:, gs],
                            start=False,
                            stop=False,
                        )

                    if ko == 0:
                        t13(True, False)
                        t2()
                    elif ko == KO - 1:
                        t2()
                        t13(False, True)
                    else:
                        t2()
                        t13(False, False)
                for tt in range(NT):
                    ts = slice(tt * TTILE, (tt + 1) * TTILE)
                    nc.tensor.matmul(
                        h_t[tt][:],
                        lhsT=arsb[:, ko, :],
                        rhs=xh_ko[:, ts],
                        start=(ko == 0),
                        stop=(ko == KO - 1),
                    )
                # fp8 cast of this chunk, split DVE/ACT (both engines
                # idle in phase 1; all their DMA issues precede the
                # casts in their instruction streams)
                if ko % 2 == 0:
                    nc.vector.tensor_copy(x8sb[:, ko, :], xh_ko[:])
                else:
                    nc.scalar.activation(x8sb[:, ko, :], xh_ko[:], Act.Copy)
                # early base DR step after each odd chunk's cast
                if ko % 2 == 1:
                    k2 = ko // 2
                    for tt in range(2):
                        ts = slice(tt * TTILE, (tt + 1) * TTILE)
                        nc.tensor.matmul(
                            acc_e[tt][:],
                            lhsT=w_sb[0][:, k2, :, :],
                            rhs=x8sb[:, 2 * k2 : 2 * k2 + 2, ts],
                            start=(k2 == 0),
                            stop=False,
                            perf_mode=DR,
                        )

            # ---- gating: top-2 softmax -> per-rank gates ----
            # copy both logits tiles out of PSUM first, then free their
            # banks for two more base groups (ot=1) that fill the PE
            # while the DVE runs the gating chain
            lgs16 = []
            for g in range(NGT):
                t_ = gp.tile([2 * E, GT], f32, tag="lgs", name=f"lgs{g}")
                nc.vector.tensor_copy(t_[:], lgs_t[g][:])
                lgs16.append(t_)
            stack_l.close()
            ps_e2 = phase1.enter_context(
                tc.tile_pool(name="ps_e2", bufs=2, space="PSUM")
            )
            acc_e2 = [
                ps_e2.tile([P, TTILE], f32, tag="acce2", name=f"acce2{t}")
                for t in range(2)
            ]

            def dr_fill(acc, ot, tt, k2s):
                ts = slice(tt * TTILE, (tt + 1) * TTILE)
                for k2 in k2s:
                    nc.tensor.matmul(
                        acc[:],
                        lhsT=w_sb[ot][:, k2, :, :],
                        rhs=x8sb[:, 2 * k2 : 2 * k2 + 2, ts],
                        start=(k2 == 0),
                        stop=False,
                        perf_mode=DR,
                    )

            # token-major transpose of the [16, GT] logits (both tiles)
            ltk16s = []
            for g in range(NGT):
                ltk16 = gp.tile([P, NGC, 2 * E], f32, tag="ltk16", name=f"lt16{g}")
                for c in range(NGC):
                    tp = ps_t.tile([P, GT], f32, tag="pt", name="tp")[:, : 2 * E]
                    nc.tensor.transpose(
                        tp[:], lgs16[g][:, c * P : (c + 1) * P],
                        idsb[: 2 * E, : 2 * E],
                    )
                    nc.vector.tensor_copy(ltk16[:, c, :], tp[:])
                ltk16s.append(ltk16)

            # PE filler while the DVE top-2 chain runs
            dr_fill(acc_e2[0], 1, 0, range(KO2))

            gates = []
            for g in range(NGT):
                ltk16 = ltk16s[g]
                ltk = gp.tile([P, NGC, E], f32, tag="ltk", name=f"ltk{g}")
                nc.vector.tensor_tensor(
                    ltk[:], ltk16[:, :, :E], ltk16[:, :, E:], Alu.add
                )
                m1 = gp.tile([P, NGC, 1], f32, tag="m1")
                nc.vector.tensor_reduce(m1[:], ltk[:], mybir.AxisListType.X, Alu.max)
                mask1 = gp.tile([P, NGC, E], f32, tag="mask1")
                nc.vector.tensor_tensor(
                    mask1[:], ltk[:], m1.to_broadcast((P, NGC, E)), Alu.is_equal
                )
                l2 = gp.tile([P, NGC, E], f32, tag="l2")
                nc.vector.scalar_tensor_tensor(
                    l2[:], mask1[:], -1e30, ltk[:], Alu.mult, Alu.add
                )
                m2 = gp.tile([P, NGC, 1], f32, tag="m2")
                nc.vector.tensor_reduce(m2[:], l2[:], mybir.AxisListType.X, Alu.max)
                mask2 = gp.tile([P, NGC, E], f32, tag="mask2")
                nc.vector.tensor_tensor(
                    mask2[:], l2[:], m2.to_broadcast((P, NGC, E)), Alu.is_equal
                )
                dlt = gp.tile([P, NGC, 1], f32, tag="dlt")
                nc.vector.tensor_tensor(dlt[:], m2[:], m1[:], Alu.subtract)
                dlts = gp.tile([P, NGC, 1], f32, tag="dlts")
                nc.vector.tensor_scalar(
                    dlts[:], dlt[:], 1.0 / 64.0, 0.0, Alu.mult, Alu.add
                )
                g2 = gp.tile([P, NGC, 1], f32, tag="g2")
                nc.scalar.activation(g2[:], dlts[:], Act.Sigmoid)
                g1 = gp.tile([P, NGC, 1], f32, tag="g1")
                nc.vector.tensor_scalar(g1[:], g2[:], -1.0, 1.0, Alu.mult, Alu.add)

                gate = gp.tile([P, NGC, E], f32, tag="gate", name=f"gate{g}")
                nc.vector.tensor_tensor(
                    gate[:], mask1[:], g1.to_broadcast((P, NGC, E)), Alu.mult
                )
                gm2 = gp.tile([P, NGC, E], f32, tag="gm2")
                nc.vector.tensor_tensor(
                    gm2[:], mask2[:], g2.to_broadcast((P, NGC, E)), Alu.mult
                )
                nc.vector.tensor_tensor(gate[:], gate[:], gm2[:], Alu.add)
                gates.append(gate)

            # second PE filler group
            dr_fill(acc_e2[1], 1, 1, range(KO2))

            for g in range(NGT):
                gs = slice(g * GT, (g + 1) * GT)
                # transpose gates back to expert-major [8, 512]
                gts = gp.tile([E, GT], f32, tag="gts", name=f"gts{g}")
                for c in range(NGC):
                    tp2 = ps_t.tile([P, GT], f32, tag="pt", name="tp2")[:E, :P]
                    nc.tensor.transpose(tp2[:], gates[g][:, c, :], idsb[:])
                    nc.vector.tensor_copy(gts[:, c * P : (c + 1) * P], tp2[:])

                # expand expert gates (x scaling*64, folded into e8) to
                # the 128 rank slots: RG = e8.T @ gts
                RG = ps_t.tile([P, GT], f32, tag="pt", name="RG")
                nc.tensor.matmul(
                    RG[:], lhsT=e8sb[:], rhs=gts[:], start=True, stop=True
                )
                nc.vector.tensor_copy(rgp[:, gs], RG[:])
                # gated rank activations for this token tile (g == tt)
                nc.vector.tensor_tensor(
                    hwsb[:, gs], h_t[g][:], rgp[:, gs], Alu.mult
                )

            # ---- close the four early groups: lora term + epilogue ----
            for ot, accs in ((0, acc_e), (1, acc_e2)):
                os_ = slice(ot * P, (ot + 1) * P)
                for tt in range(2):
                    ts = slice(tt * TTILE, (tt + 1) * TTILE)
                    nc.tensor.matmul(
                        accs[tt][:],
                        lhsT=lbsb[:, os_],
                        rhs=hwsb[:, ts],
                        start=False,
                        stop=True,
                    )
                    osb = ob.tile([P, TTILE], f16, tag="osb", name="osbe")
                    nc.vector.scalar_tensor_tensor(
                        osb[:],
                        accs[tt][:],
                        1.0 / W8_SCALE,
                        bbsb[:, ot, None].to_broadcast((P, TTILE)),
                        Alu.mult,
                        Alu.add,
                    )
                    (nc.sync if tt == 0 else nc.scalar).dma_start(
                        yt_v[:, ot, ts], osb[:]
                    )

            phase1.close()

            # ---- phase 2: remaining base matmul + fused lora_B ----
            phase2 = ExitStack()
            ps_o = phase2.enter_context(
                tc.tile_pool(name="ps_o", bufs=6, space="PSUM")
            )

            for ot in range(2, OTILES):
                # keep the weight stream ~3 tiles ahead, alternating queues
                pre = ot + 3
                if pre < OTILES and w_sb[pre] is None:
                    w_sb[pre] = wpool.tile(
                        [P, KO2, 2, P], f8, tag="w", name=f"w{pre}"
                    )
                    eng = nc.scalar if pre % 2 == 0 else nc.gpsimd
                    eng.dma_start(w_sb[pre][:], wt_v[:, pre, :, :, :])
                os_ = slice(ot * P, (ot + 1) * P)
                for tt in range(NT):
                    ts = slice(tt * TTILE, (tt + 1) * TTILE)
                    acc = ps_o.tile([P, TTILE], f32, tag="acc")
                    for k2 in range(KO2):
                        nc.tensor.matmul(
                            acc[:],
                            lhsT=w_sb[ot][:, k2, :, :],
                            rhs=x8sb[:, 2 * k2 : 2 * k2 + 2, ts],
                            start=(k2 == 0),
                            stop=False,
                            perf_mode=DR,
                        )
                    nc.tensor.matmul(
                        acc[:],
                        lhsT=lbsb[:, os_],
                        rhs=hwsb[:, ts],
                        start=False,
                        stop=True,
                    )
                    osb = ob.tile([P, TTILE], f16, tag="osb")
                    # acc holds 64x(base+lora); rescale + bias in one op
                    nc.vector.scalar_tensor_tensor(
                        osb[:],
                        acc[:],
                        1.0 / W8_SCALE,
                        bbsb[:, ot, None].to_broadcast((P, TTILE)),
                        Alu.mult,
                        Alu.add,
                    )
                    (nc.sync if tt == 0 else nc.scalar).dma_start(
                        yt_v[:, ot, ts], osb[:]
                    )
            phase2.close()

    nc.compile()
    return nc


def get_program():
    if "nc" not in _prog_cache:
        _prog_cache["nc"] = _build_program()
    return _prog_cache["nc"]


def make_in_maps(x, base_w, base_b, lora_A, lora_B, router_w, scalings):
    """Host-side sharding/layout prep -> per-core input dicts."""
    import ml_dtypes

    x = np.ascontiguousarray(x, dtype=np.float32)
    # partition-major layout [P, KO, T]: per-core 4-ko DMA blocks are
    # 1MB with 8KB contiguous runs per partition
    xt_full = np.ascontiguousarray(
        x.reshape(T, KO, P).transpose(2, 1, 0)
    )  # [P, KO, T]

    # base weights x64 -> e4m3, DoubleRow pair layout [ot,p,k2,j,f]
    wt_host = np.ascontiguousarray(
        (base_w.T.astype(np.float32) * W8_SCALE)
        .reshape(KO2, 2, P, OTILES, P)
        .transpose(3, 2, 0, 1, 4)
        .reshape(OTILES * P, KO2 * 2 * P)
        .astype(ml_dtypes.float8_e4m3)
    )
    lb_host = np.ascontiguousarray(
        lora_B.T.astype(np.float32).astype(ml_dtypes.bfloat16)
    )

    # lora_A.T (unscaled; scaling folded into e8) -> [p, ko*128+r]
    ar_host = np.ascontiguousarray(
        lora_A.T.astype(np.float32)
        .reshape(KO, P, R)
        .transpose(1, 0, 2)
        .reshape(P, KO * R)
        .astype(np.float16)
    )

    # router_w.T x64 -> [p, ko, 16]: cols 0:8 = fp16 hi, 8:16 = fp16 lo
    # (hi + lo == 64*rw to ~2^-24 relative)
    rw64 = np.ascontiguousarray(
        router_w.T.astype(np.float32)
        .reshape(KO, P, E)
        .transpose(1, 0, 2)
    ) * np.float32(64.0)                                  # [P, KO, E]
    rwh_host = rw64.astype(np.float16)
    rwr_host = (rw64 - rwh_host.astype(np.float32)).astype(np.float16)
    rw2_host = np.ascontiguousarray(
        np.concatenate([rwh_host, rwr_host], axis=-1).reshape(P, KO * 2 * E)
    )

    # expert -> rank-slot expansion with per-expert scaling and the x64
    # fp8 weight scale folded in (so the lora matmul accumulates at the
    # same scale as the fp8 base steps)
    e8 = np.zeros((E, P), dtype=np.float32)
    s = np.asarray(scalings, dtype=np.float32) * W8_SCALE
    for e in range(E):
        e8[e, e * RANK : (e + 1) * RANK] = s[e]
    idm = np.eye(P, dtype=np.float32)
    bbf = base_b.astype(np.float32)

    xh_full = xt_full.astype(np.float16)
    rh_full = (xt_full - xh_full.astype(np.float32)).astype(np.float16)
    # pack hi/lo per-ko: [P, KO, 2, T]
    xr_full = np.stack([xh_full, rh_full], axis=2)

    in_maps = []
    for c in range(N_CORES):
        cs = slice(c * T_PC, (c + 1) * T_PC)
        m = {
            "xr": np.ascontiguousarray(xr_full[:, :, :, cs]).reshape(
                P, KO * 2 * T_PC
            ),
            "wt": wt_host,
            "lb": lb_host,
            "ar": ar_host,
            "rw2": rw2_host,
            "bb": bbf,
            "e8": e8,
            "idm": idm,
        }
        in_maps.append(m)
    return in_maps


def assemble_output(results):
    """Per-core yt [O, T_PC] fp16 -> full [B, S, O] f32."""
    yt_full = np.concatenate(
        [np.asarray(r["yt"]) for r in results], axis=1
    )  # [O, T] fp16
    return np.ascontiguousarray(yt_full.T.astype(np.float32)).reshape(B, S, O)


def kernel(**inputs):
    _ensure_path()
    from concourse.bass_utils import run_bass_kernel_spmd

    assert int(inputs["top_k"]) == 2
    nc = get_program()
    in_maps = make_in_maps(
        inputs["x"],
        inputs["base_w"],
        inputs["base_b"],
        inputs["lora_A"],
        inputs["lora_B"],
        inputs["router_w"],
        inputs["scalings"],
    )
    res = run_bass_kernel_spmd(nc, in_maps, list(range(N_CORES)))
    return assemble_output(res.results)


if __name__ == "__main__":
    # quick smoke: build the program only
    get_program()
    print("program built OK")
